# revision 10
# baseline (speedup 1.0000x reference)
"""MoE (dropless, top-2 of 8 experts, GLU erf-gelu MLP) Trainium2 kernel.

Expert-parallel across 8 NeuronCores with HOST-side routing/dispatch/combine:
the router (softmax + top-2) runs in f64 numpy inside kernel(), tokens are
compacted per expert on the host, and the device program is a pure dense GLU
MLP over each expert's compacted token block.

Device math uses error-compensated fp8 (e4m3) matmuls in DoubleRow perf mode
(two 128-deep contraction tiles per instruction):
  a·b  ~=  a_hi·b_hi + a_lo·b_hi + a_hi·b_lo
with a_hi = fp8(a·s), a_lo = fp8(a·s - a_hi). Both the h-matmuls (x·w1, x·v1,
split on host) and the y-matmul (h·w2; h split on device, w2 split on host)
use the 3-term form, giving ~bf16 accuracy at 0.75x the bf16 PE cost.

Per core (expert e, C = max expert load tokens, zero-padded columns):
  - x^T hi/lo fp8 [2, DO, P, C] streams in once, weight chunks stream per
    512-wide F-chunk; ph1/ph2 accumulate 3-term DoubleRow products in PSUM.
  - ACT applies erf-gelu (descale 1/(SX*SW) folded into the activation
    scale); DVE forms h = gelu(h1)*h2 scaled by SH, then splits h into
    fp8 hi/lo for the y-matmul.
  - y accumulates over F in PSUM per 1024-wide F-group, and group partials
    add into an SBUF f32 accumulator; the final group writes bf16 and DMAs
    a compact [C, D] output (scaled by SH*SW2; host descales).
The host combine does out[tok] += w_tok * y_row / (SH*SW2) + bias.

Self-contained: hardcodes all shapes (x [2,2048,1024], E=8, F=2816, top-2).
"""

import os
import sys

import numpy as np

for _p in ("/opt/trn_rl_repo", "/root/.axon_site/_ro/trn_rl_repo"):
    if os.path.isdir(_p) and _p not in sys.path:
        sys.path.append(_p)

import concourse.bass as bass  # noqa: E402
import concourse.bacc as bacc  # noqa: E402
import concourse.mybir as mybir  # noqa: E402
import concourse.tile as tile  # noqa: E402
from concourse.bass import ds, ts  # noqa: E402

F32 = mybir.dt.float32
FP8 = mybir.dt.float8e4
BF16 = mybir.dt.bfloat16
AF = mybir.ActivationFunctionType
OP = mybir.AluOpType
DR = mybir.MatmulPerfMode.DoubleRow

P = 128
T = 4096          # tokens (2*2048)
D = 1024          # model dim
F = 2816          # ffn dim
E = 8             # experts
TOP_K = 2
DO = D // P       # 8 d-blocks
NU = F // P       # 22 f-subtiles
FCH = 512         # F chunk (DMA + h-compute granularity)
GCH = 2           # chunks per y PSUM accumulation group

SX = 32.0         # x scale before fp8 split
SW = 2048.0       # w1/v1 scale
SH = 16.0         # h scale (device-side split)
SW2 = 2048.0      # w2 scale
FP8MAX = 240.0    # ml_dtypes float8_e4m3 max finite

TERMS = ((0, 0), (1, 0), (0, 1))   # (w_half, x_half): hi*hi, lo*hi, hi*lo


def _blocks(total, step):
    out = []
    o = 0
    while o < total:
        s = min(step, total - o)
        out.append((o, s))
        o += s
    return out


def build_nc(C):
    NJ = (C + P - 1) // P          # token tiles for y
    CP = NJ * P                    # padded token stride (dual-fp8 alignment)
    chunks = _blocks(F, FCH)       # [(c0, fc)]
    nc = bacc.Bacc()

    x_d = nc.dram_tensor("x8", [2, DO, P, CP], FP8, kind="ExternalInput")
    w1_d = nc.dram_tensor("w1", [2, DO, P, F], FP8, kind="ExternalInput")
    v1_d = nc.dram_tensor("v1", [2, DO, P, F], FP8, kind="ExternalInput")
    w2_d = nc.dram_tensor("w2", [2, NU, P, D], FP8, kind="ExternalInput")
    yc_d = nc.dram_tensor("yc", [C, D], BF16, kind="ExternalOutput")

    with tile.TileContext(nc) as tc:
        with (
            tc.tile_pool(name="persist", bufs=1) as pp,
            tc.tile_pool(name="wts", bufs=2) as wpool,
            tc.tile_pool(name="w2p", bufs=2) as w2pool,
            tc.tile_pool(name="hsp", bufs=2) as hpool,
            tc.tile_pool(name="scr", bufs=3) as gpool,
            tc.tile_pool(name="psA", bufs=2, space="PSUM") as psA,
            tc.tile_pool(name="psY", bufs=3, space="PSUM") as psY,
        ):
            x_sb = pp.tile([P, 2, DO, CP], FP8)
            y_sb = pp.tile([P, NJ, D], F32)
            y_out = pp.tile([P, NJ, D], BF16)

            tb = _blocks(C, 512)       # token blocks (PSUM bank width)

            n_groups = (len(chunks) + GCH - 1) // GCH
            # per group: list of (chunk_index, u_base_in_group, fc)
            groups = []
            for g in range(n_groups):
                cs = chunks[g * GCH : (g + 1) * GCH]
                groups.append((g * GCH, cs))

            hh = {}
            hl = {}
            w2g = {}

            def emit_h_chunk(ci):
                c0, fc = chunks[ci]
                g = ci // GCH
                w1c = wpool.tile([P, 2, DO, FCH], FP8, tag="w1", name="w1c")
                v1c = wpool.tile([P, 2, DO, FCH], FP8, tag="v1", name="v1c")
                if ci == 0:
                    # startup-critical: stream pieces in first-use order so
                    # the PE starts as soon as w1 hi + the first x hi pair land
                    nc.sync.dma_start(
                        w1c[:, 0, :, :fc],
                        w1_d[0, :, :, ds(c0, fc)].rearrange("o p f -> p o f"),
                    )
                    for xi in range(2):
                        for j4 in range(DO // 2):
                            nc.sync.dma_start(
                                x_sb[:, xi, ts(j4, 2), :],
                                x_d[xi, ds(2 * j4, 2)].rearrange("o p c -> p o c"),
                            )
                    nc.sync.dma_start(
                        w1c[:, 1, :, :fc],
                        w1_d[1, :, :, ds(c0, fc)].rearrange("o p f -> p o f"),
                    )
                    for wi in range(2):
                        nc.sync.dma_start(
                            v1c[:, wi, :, :fc],
                            v1_d[wi, :, :, ds(c0, fc)].rearrange("o p f -> p o f"),
                        )
                else:
                    nc.sync.dma_start(
                        w1c[:, :, :, :fc],
                        w1_d[:, :, :, ds(c0, fc)].rearrange("h o p f -> p h o f"),
                    )
                    nc.sync.dma_start(
                        v1c[:, :, :, :fc],
                        v1_d[:, :, :, ds(c0, fc)].rearrange("h o p f -> p h o f"),
                    )
                if ci % GCH == 0:
                    # new group: h split buffers
                    hh[g] = hpool.tile([P, GCH * FCH // P, CP], FP8, tag="hh", name="hh")
                    hl[g] = hpool.tile([P, GCH * FCH // P, CP], FP8, tag="hl", name="hl")
                if ci % GCH == 1 or ci == n_chunks - 1:
                    # group's w2 slice: deferred off the startup/chunk-head
                    # DMA critical path (first needed by y(g) much later)
                    cg0 = chunks[g * GCH][0]
                    u0 = cg0 // P
                    nug = min(GCH * FCH, F - cg0) // P
                    w2g[g] = w2pool.tile([P, 2, GCH * FCH // P, D], FP8, name="w2g")
                    for wi in range(2):
                        nc.sync.dma_start(
                            w2g[g][:, wi, :nug, :],
                            w2_d[wi, ds(u0, nug), :, :].rearrange("u p d -> p u d"),
                        )
                def mm_group(ph, wgt, u, b0, bs, terms):
                    sub = _blocks(bs, 256)
                    nmm = len(sub) * 4 * 3
                    i = 0
                    for (wi, xi) in terms:
                        for (s0, sn) in sub:
                            for j4 in range(DO // 2):
                                nc.tensor.matmul(
                                    ph[:, ds(s0, sn)],
                                    wgt[:, wi, ts(j4, 2), ts(u, P)],
                                    x_sb[:, xi, ts(j4, 2), ds(b0 + s0, sn)],
                                    start=(i == 0),
                                    stop=(i == nmm - 1),
                                    perf_mode=DR,
                                )
                                i += 1

                def split_h(ph2_t, g_ap, uu, b0, bs):
                    ht = gpool.tile([P, 512], F32, tag="ht", name="ht")
                    nc.vector.scalar_tensor_tensor(
                        ht[:, :bs], ph2_t[:, :bs], SH / (SX * SW), g_ap,
                        op0=OP.mult, op1=OP.mult,
                    )
                    nc.scalar.copy(hh[g][:, uu, ds(b0, bs)], ht[:, :bs])
                    nc.vector.tensor_tensor(
                        hl[g][:, uu, ds(b0, bs)], ht[:, :bs],
                        hh[g][:, uu, ds(b0, bs)], op=OP.subtract,
                    )

                if ci == 0:
                    # two-phase: all ph1+gelu (no v1 dependency) first, g
                    # parked in bf16; ph2+split once v1 lands — keeps the PE
                    # fed while the back half of the startup DMAs stream
                    t0 = ((0, 0), (0, 1), (1, 0))   # x_lo before w1_lo
                    gall = pp.tile([P, FCH // P, CP], BF16)
                    for u in range(fc // P):
                        for (b0, bs) in tb:
                            ph1 = psA.tile([P, 512], F32, tag="h1", name="ph1")
                            mm_group(ph1, w1c, u, b0, bs, t0)
                            nc.scalar.activation(
                                gall[:, u, ds(b0, bs)], ph1[:, :bs], AF.Gelu,
                                scale=1.0 / (SX * SW),
                            )
                    for u in range(fc // P):
                        for (b0, bs) in tb:
                            ph2 = psA.tile([P, 512], F32, tag="h2", name="ph2")
                            mm_group(ph2, v1c, u, b0, bs, t0)
                            split_h(ph2, gall[:, u, ds(b0, bs)], u, b0, bs)
                else:
                    for u in range(fc // P):
                        uu = (ci % GCH) * (FCH // P) + u
                        for (b0, bs) in tb:
                            ph1 = psA.tile([P, 512], F32, tag="h1", name="ph1")
                            ph2 = psA.tile([P, 512], F32, tag="h2", name="ph2")
                            mm_group(ph1, w1c, u, b0, bs, TERMS)
                            mm_group(ph2, v1c, u, b0, bs, TERMS)
                            g_t = gpool.tile([P, 512], F32, tag="g", name="g_t")
                            nc.scalar.activation(
                                g_t[:, :bs], ph1[:, :bs], AF.Gelu,
                                scale=1.0 / (SX * SW),
                            )
                            split_h(ph2, g_t[:, :bs], uu, b0, bs)

            def emit_y_group(g):
                ci0, cs = groups[g]
                nug = sum(fc for _, fc in cs) // P
                last = g == len(groups) - 1
                # last group: big j tiles first so the final add+DMA tail is
                # the smallest tile; per-dh DMAs overlap with remaining work
                jorder = range(NJ) if not last else sorted(
                    range(NJ), key=lambda j: -min(P, C - j * P)
                )
                for j in jorder:
                    jn = min(P, C - j * P)
                    for dh in range(2):
                        py = psY.tile([P, 512], F32, tag="y", name="py")
                        nmm = 2 * (nug // 2) * 3
                        i = 0
                        for db in range(2):
                            for up in range(nug // 2):
                                for (wi, hi_) in TERMS:
                                    hsp = hh[g] if hi_ == 0 else hl[g]
                                    nc.tensor.matmul(
                                        py[:jn, ds(db * 256, 256)],
                                        hsp[:, ts(up, 2), ds(j * P, jn)],
                                        w2g[g][:, wi, ts(up, 2), ds(dh * 512 + db * 256, 256)],
                                        start=(i == 0),
                                        stop=(i == nmm - 1),
                                        perf_mode=DR,
                                    )
                                    i += 1
                        if g == 0:
                            nc.vector.tensor_copy(
                                y_sb[:jn, j, ts(dh, 512)], py[:jn, :]
                            )
                        elif not last:
                            nc.vector.tensor_tensor(
                                y_sb[:jn, j, ts(dh, 512)], py[:jn, :],
                                y_sb[:jn, j, ts(dh, 512)], op=OP.add,
                            )
                        else:
                            nc.vector.tensor_tensor(
                                y_out[:jn, j, ts(dh, 512)], py[:jn, :],
                                y_sb[:jn, j, ts(dh, 512)], op=OP.add,
                            )
                            nc.sync.dma_start(
                                yc_d[ds(j * P, jn), ts(dh, 512)],
                                y_out[:jn, j, ts(dh, 512)],
                            )

            # pipeline: y(g) emitted one chunk after its group completes so
            # the gelu/split chain hides under the next chunk's PE work
            n_chunks = len(chunks)
            emitted = 0
            for ci in range(n_chunks):
                emit_h_chunk(ci)
                done = (ci + 1) // GCH   # groups fully computed so far
                if (ci + 1) % GCH == 1 and emitted < done:
                    emit_y_group(emitted)
                    emitted += 1
            while emitted < n_groups:
                emit_y_group(emitted)
                emitted += 1

    nc.finalize()
    return nc


def _split_fp8(a, scale, np_fp8):
    s = (a.astype(np.float32) * np.float32(scale))
    hi = np.clip(s, -FP8MAX, FP8MAX).astype(np_fp8)
    lo = np.clip(s - hi.astype(np.float32), -FP8MAX, FP8MAX).astype(np_fp8)
    return hi, lo


def _route(x2d, rw):
    """f64 router: softmax + top-2 (ties -> lower index, like lax.top_k)."""
    logits = x2d.astype(np.float64) @ rw.astype(np.float64)
    m = logits.max(axis=-1, keepdims=True)
    p = np.exp(logits - m)
    p /= p.sum(axis=-1, keepdims=True)
    idx = np.argsort(-p, axis=-1, kind="stable")[:, :TOP_K]
    wts = np.take_along_axis(p, idx, axis=1)
    return idx.astype(np.int64), wts.astype(np.float32)


def make_in_maps(inputs, idx, C):
    import ml_dtypes

    np_fp8 = ml_dtypes.float8_e4m3

    x = np.asarray(inputs["x"], dtype=np.float32).reshape(T, D)
    w1 = np.asarray(inputs["w1"], dtype=np.float32)
    v1 = np.asarray(inputs["v1"], dtype=np.float32)
    w2 = np.asarray(inputs["w2"], dtype=np.float32)

    in_maps = []
    toks = []
    for e in range(E):
        tok = np.where((idx == e).any(axis=1))[0]
        toks.append(tok)
        CP = ((C + P - 1) // P) * P
        xg = np.zeros((CP, D), dtype=np.float32)
        xg[: len(tok)] = x[tok]
        xh, xl = _split_fp8(xg.T, SX, np_fp8)            # [D, CP]
        x8 = np.ascontiguousarray(
            np.stack([xh, xl]).reshape(2, DO, P, CP)
        )
        w1h, w1l = _split_fp8(w1[e], SW, np_fp8)         # [D, F]
        v1h, v1l = _split_fp8(v1[e], SW, np_fp8)
        w2h, w2l = _split_fp8(w2[e], SW2, np_fp8)        # [F, D]
        in_maps.append(
            {
                "x8": x8,
                "w1": np.ascontiguousarray(np.stack([w1h, w1l]).reshape(2, DO, P, F)),
                "v1": np.ascontiguousarray(np.stack([v1h, v1l]).reshape(2, DO, P, F)),
                "w2": np.ascontiguousarray(np.stack([w2h, w2l]).reshape(2, NU, P, D)),
            }
        )
    return in_maps, toks


_NC_CACHE = {}
last_results = None


def kernel(**inputs) -> np.ndarray:
    global last_results
    from concourse.bass_utils import run_bass_kernel_spmd

    x2d = np.asarray(inputs["x"], dtype=np.float32).reshape(T, D)
    rw = np.asarray(inputs["router_w"], dtype=np.float32)
    bias = np.asarray(inputs["bias"], dtype=np.float32)

    idx, wts = _route(x2d, rw)
    counts = np.bincount(idx.ravel(), minlength=E)
    C = int(counts.max())

    key = ("nc", C)
    if key not in _NC_CACHE:
        _NC_CACHE[key] = build_nc(C)
        _NC_CACHE["nc"] = _NC_CACHE[key]
    nc = _NC_CACHE[key]

    in_maps, toks = make_in_maps(inputs, idx, C)
    trace = bool(int(os.environ.get("MOE_TRACE", "0")))
    res = run_bass_kernel_spmd(
        nc, in_maps, core_ids=list(range(E)), trace=trace,
        stitch_traces=trace, trace_cores=list(range(E)) if trace else None,
    )
    last_results = res

    descale = np.float32(1.0 / (SH * SW2))
    out = np.zeros((T, D), dtype=np.float32)
    for e in range(E):
        tok = toks[e]
        yc = np.asarray(res.results[e]["yc"]).astype(np.float32)[: len(tok)]
        we = np.where(idx[tok, 0] == e, wts[tok, 0], wts[tok, 1])
        out[tok] += we[:, None] * (yc * descale)
    out += bias
    return out.reshape(2, 2048, D)


# revision 12
# speedup vs baseline: 1.0094x; 1.0094x over previous
"""MoE (dropless, top-2 of 8 experts, GLU erf-gelu MLP) Trainium2 kernel.

Expert-parallel across 8 NeuronCores with HOST-side routing/dispatch/combine:
the router (softmax + top-2) runs in f64 numpy inside kernel(), tokens are
compacted per expert on the host, and the device program is a pure dense GLU
MLP over each expert's compacted token block.

Device math uses error-compensated fp8 (e4m3) matmuls in DoubleRow perf mode
(two 128-deep contraction tiles per instruction):
  a·b  ~=  a_hi·b_hi + a_lo·b_hi + a_hi·b_lo
with a_hi = fp8(a·s), a_lo = fp8(a·s - a_hi). Both the h-matmuls (x·w1, x·v1,
split on host) and the y-matmul (h·w2; h split on device, w2 split on host)
use the 3-term form, giving ~bf16 accuracy at 0.75x the bf16 PE cost.

Per core (expert e, C = max expert load tokens, zero-padded columns):
  - x^T hi/lo fp8 [2, DO, P, C] streams in once, weight chunks stream per
    512-wide F-chunk; ph1/ph2 accumulate 3-term DoubleRow products in PSUM.
  - ACT applies erf-gelu (descale 1/(SX*SW) folded into the activation
    scale); DVE forms h = gelu(h1)*h2 scaled by SH, then splits h into
    fp8 hi/lo for the y-matmul.
  - y accumulates over F in PSUM per 1024-wide F-group, and group partials
    add into an SBUF f32 accumulator; the final group writes bf16 and DMAs
    a compact [C, D] output (scaled by SH*SW2; host descales).
The host combine does out[tok] += w_tok * y_row / (SH*SW2) + bias.

Self-contained: hardcodes all shapes (x [2,2048,1024], E=8, F=2816, top-2).
"""

import os
import sys

import numpy as np

for _p in ("/opt/trn_rl_repo", "/root/.axon_site/_ro/trn_rl_repo"):
    if os.path.isdir(_p) and _p not in sys.path:
        sys.path.append(_p)

import concourse.bass as bass  # noqa: E402
import concourse.bacc as bacc  # noqa: E402
import concourse.mybir as mybir  # noqa: E402
import concourse.tile as tile  # noqa: E402
from concourse.bass import ds, ts  # noqa: E402

F32 = mybir.dt.float32
FP8 = mybir.dt.float8e4
BF16 = mybir.dt.bfloat16
AF = mybir.ActivationFunctionType
OP = mybir.AluOpType
DR = mybir.MatmulPerfMode.DoubleRow

P = 128
T = 4096          # tokens (2*2048)
D = 1024          # model dim
F = 2816          # ffn dim
E = 8             # experts
TOP_K = 2
DO = D // P       # 8 d-blocks
NU = F // P       # 22 f-subtiles
FCH = 512         # F chunk (DMA + h-compute granularity)
GCH = 2           # chunks per y PSUM accumulation group

SX = 32.0         # x scale before fp8 split
SW = 2048.0       # w1/v1 scale
SH = 16.0         # h scale (device-side split)
SW2 = 2048.0      # w2 scale
FP8MAX = 240.0    # ml_dtypes float8_e4m3 max finite

TERMS = ((0, 0), (1, 0), (0, 1))   # (w_half, x_half): hi*hi, lo*hi, hi*lo


def _blocks(total, step):
    out = []
    o = 0
    while o < total:
        s = min(step, total - o)
        out.append((o, s))
        o += s
    return out


def build_nc(C):
    NJ = (C + P - 1) // P          # token tiles for y
    CP = NJ * P                    # padded token stride (dual-fp8 alignment)
    chunks = _blocks(F, FCH)       # [(c0, fc)]
    nc = bacc.Bacc()

    x_d = nc.dram_tensor("x8", [2, DO, P, CP], FP8, kind="ExternalInput")
    w1_d = nc.dram_tensor("w1", [2, DO, P, F], FP8, kind="ExternalInput")
    v1_d = nc.dram_tensor("v1", [2, DO, P, F], FP8, kind="ExternalInput")
    w2_d = nc.dram_tensor("w2", [2, NU, P, D], FP8, kind="ExternalInput")
    yc_d = nc.dram_tensor("yc", [C, D], BF16, kind="ExternalOutput")

    with tile.TileContext(nc) as tc:
        with (
            tc.tile_pool(name="persist", bufs=1) as pp,
            tc.tile_pool(name="wts", bufs=2) as wpool,
            tc.tile_pool(name="w2p", bufs=2) as w2pool,
            tc.tile_pool(name="hsp", bufs=2) as hpool,
            tc.tile_pool(name="scr", bufs=3) as gpool,
            tc.tile_pool(name="psA", bufs=2, space="PSUM") as psA,
            tc.tile_pool(name="psY", bufs=2, space="PSUM") as psY,
        ):
            x_sb = pp.tile([P, 2, DO, CP], FP8)
            y_sb = pp.tile([P, NJ, D], F32)
            y_out = pp.tile([P, NJ, D], BF16)

            tb = _blocks(C, 512)       # token blocks (PSUM bank width)

            n_groups = (len(chunks) + GCH - 1) // GCH
            # per group: list of (chunk_index, u_base_in_group, fc)
            groups = []
            for g in range(n_groups):
                cs = chunks[g * GCH : (g + 1) * GCH]
                groups.append((g * GCH, cs))

            hh = {}
            hl = {}
            w2g = {}

            def emit_h_chunk(ci):
                c0, fc = chunks[ci]
                g = ci // GCH
                w1c = wpool.tile([P, 2, DO, FCH], FP8, tag="w1", name="w1c")
                v1c = wpool.tile([P, 2, DO, FCH], FP8, tag="v1", name="v1c")
                if ci == 0:
                    # startup-critical: stream pieces in first-use order so
                    # the PE starts as soon as w1 hi + the first x hi pair land
                    nc.sync.dma_start(
                        w1c[:, 0, :, :fc],
                        w1_d[0, :, :, ds(c0, fc)].rearrange("o p f -> p o f"),
                    )
                    for xi in range(2):
                        for j4 in range(DO // 2):
                            nc.sync.dma_start(
                                x_sb[:, xi, ts(j4, 2), :],
                                x_d[xi, ds(2 * j4, 2)].rearrange("o p c -> p o c"),
                            )
                    nc.sync.dma_start(
                        w1c[:, 1, :, :fc],
                        w1_d[1, :, :, ds(c0, fc)].rearrange("o p f -> p o f"),
                    )
                    for wi in range(2):
                        nc.sync.dma_start(
                            v1c[:, wi, :, :fc],
                            v1_d[wi, :, :, ds(c0, fc)].rearrange("o p f -> p o f"),
                        )
                else:
                    nc.sync.dma_start(
                        w1c[:, :, :, :fc],
                        w1_d[:, :, :, ds(c0, fc)].rearrange("h o p f -> p h o f"),
                    )
                    nc.sync.dma_start(
                        v1c[:, :, :, :fc],
                        v1_d[:, :, :, ds(c0, fc)].rearrange("h o p f -> p h o f"),
                    )
                if ci % GCH == 0:
                    # new group: h split buffers
                    hh[g] = hpool.tile([P, GCH * FCH // P, CP], FP8, tag="hh", name="hh")
                    hl[g] = hpool.tile([P, GCH * FCH // P, CP], FP8, tag="hl", name="hl")
                if ci % GCH == 1 or ci == n_chunks - 1:
                    # group's w2 slice: deferred off the startup/chunk-head
                    # DMA critical path (first needed by y(g) much later)
                    cg0 = chunks[g * GCH][0]
                    u0 = cg0 // P
                    nug = min(GCH * FCH, F - cg0) // P
                    w2g[g] = w2pool.tile([P, 2, GCH * FCH // P, D], FP8, name="w2g")
                    for wi in range(2):
                        nc.sync.dma_start(
                            w2g[g][:, wi, :nug, :],
                            w2_d[wi, ds(u0, nug), :, :].rearrange("u p d -> p u d"),
                        )
                def mm_group(ph, wgt, u, b0, bs, terms):
                    sub = _blocks(bs, 256)
                    nmm = len(sub) * 4 * 3
                    i = 0
                    for (wi, xi) in terms:
                        for (s0, sn) in sub:
                            for j4 in range(DO // 2):
                                nc.tensor.matmul(
                                    ph[:, ds(s0, sn)],
                                    wgt[:, wi, ts(j4, 2), ts(u, P)],
                                    x_sb[:, xi, ts(j4, 2), ds(b0 + s0, sn)],
                                    start=(i == 0),
                                    stop=(i == nmm - 1),
                                    perf_mode=DR,
                                )
                                i += 1

                def split_h(ph2_t, g_ap, uu, b0, bs):
                    ht = gpool.tile([P, 512], F32, tag="ht", name="ht")
                    nc.vector.scalar_tensor_tensor(
                        ht[:, :bs], ph2_t[:, :bs], SH / (SX * SW), g_ap,
                        op0=OP.mult, op1=OP.mult,
                    )
                    nc.scalar.copy(hh[g][:, uu, ds(b0, bs)], ht[:, :bs])
                    nc.vector.tensor_tensor(
                        hl[g][:, uu, ds(b0, bs)], ht[:, :bs],
                        hh[g][:, uu, ds(b0, bs)], op=OP.subtract,
                    )

                if ci == 0:
                    # two-phase startup: all ph1+gelu (no v1 dependency)
                    # first with g parked in bf16, then ph2+split once v1
                    # lands. Within phase 1, term-layers interleave across 6
                    # concurrent PSUM banks so the PE chews the layers whose
                    # operands have landed while later DMA pieces stream.
                    t0 = ((0, 0), (0, 1), (1, 0))   # x_lo before w1_lo
                    gall = pp.tile([P, FCH // P, CP], BF16)
                    for wu in ((0, 1), (2, 3)):
                        keys = [(u, tbi) for u in wu for tbi in range(len(tb))]
                        phs = [
                            psA.tile([P, 512], F32, tag="ph", bufs=6, name="ph1w")
                            for _ in keys
                        ]
                        for li, (wi, xi) in enumerate(t0):
                            for ki, (u, tbi) in enumerate(keys):
                                b0, bs = tb[tbi]
                                sub = _blocks(bs, 256)
                                for si, (s0, sn) in enumerate(sub):
                                    for j4 in range(DO // 2):
                                        nc.tensor.matmul(
                                            phs[ki][:, ds(s0, sn)],
                                            w1c[:, wi, ts(j4, 2), ts(u, P)],
                                            x_sb[:, xi, ts(j4, 2), ds(b0 + s0, sn)],
                                            start=(li == 0 and si == 0 and j4 == 0),
                                            stop=(
                                                li == len(t0) - 1
                                                and si == len(sub) - 1
                                                and j4 == DO // 2 - 1
                                            ),
                                            perf_mode=DR,
                                        )
                        for ki, (u, tbi) in enumerate(keys):
                            b0, bs = tb[tbi]
                            nc.scalar.activation(
                                gall[:, u, ds(b0, bs)], phs[ki][:, :bs], AF.Gelu,
                                scale=1.0 / (SX * SW),
                            )
                    for u in range(fc // P):
                        for (b0, bs) in tb:
                            ph2 = psA.tile([P, 512], F32, tag="ph", bufs=6, name="ph2w")
                            mm_group(ph2, v1c, u, b0, bs, t0)
                            split_h(ph2, gall[:, u, ds(b0, bs)], u, b0, bs)
                else:
                    for u in range(fc // P):
                        uu = (ci % GCH) * (FCH // P) + u
                        for (b0, bs) in tb:
                            ph1 = psA.tile([P, 512], F32, tag="ph", bufs=6, name="ph1")
                            ph2 = psA.tile([P, 512], F32, tag="ph", bufs=6, name="ph2")
                            mm_group(ph1, w1c, u, b0, bs, TERMS)
                            mm_group(ph2, v1c, u, b0, bs, TERMS)
                            g_t = gpool.tile([P, 512], F32, tag="g", name="g_t")
                            nc.scalar.activation(
                                g_t[:, :bs], ph1[:, :bs], AF.Gelu,
                                scale=1.0 / (SX * SW),
                            )
                            split_h(ph2, g_t[:, :bs], uu, b0, bs)

            def emit_y_group(g):
                ci0, cs = groups[g]
                nug = sum(fc for _, fc in cs) // P
                last = g == len(groups) - 1
                # last group: big j tiles first so the final add+DMA tail is
                # the smallest tile; per-dh DMAs overlap with remaining work
                jorder = range(NJ) if not last else sorted(
                    range(NJ), key=lambda j: -min(P, C - j * P)
                )
                for j in jorder:
                    jn = min(P, C - j * P)
                    for dh in range(2):
                        py = psY.tile([P, 512], F32, tag="y", name="py")
                        nmm = 2 * (nug // 2) * 3
                        i = 0
                        for db in range(2):
                            for up in range(nug // 2):
                                for (wi, hi_) in TERMS:
                                    hsp = hh[g] if hi_ == 0 else hl[g]
                                    nc.tensor.matmul(
                                        py[:jn, ds(db * 256, 256)],
                                        hsp[:, ts(up, 2), ds(j * P, jn)],
                                        w2g[g][:, wi, ts(up, 2), ds(dh * 512 + db * 256, 256)],
                                        start=(i == 0),
                                        stop=(i == nmm - 1),
                                        perf_mode=DR,
                                    )
                                    i += 1
                        if g == 0:
                            nc.vector.tensor_copy(
                                y_sb[:jn, j, ts(dh, 512)], py[:jn, :]
                            )
                        elif not last:
                            nc.vector.tensor_tensor(
                                y_sb[:jn, j, ts(dh, 512)], py[:jn, :],
                                y_sb[:jn, j, ts(dh, 512)], op=OP.add,
                            )
                        else:
                            nc.vector.tensor_tensor(
                                y_out[:jn, j, ts(dh, 512)], py[:jn, :],
                                y_sb[:jn, j, ts(dh, 512)], op=OP.add,
                            )
                            nc.sync.dma_start(
                                yc_d[ds(j * P, jn), ts(dh, 512)],
                                y_out[:jn, j, ts(dh, 512)],
                            )

            # pipeline: y(g) emitted one chunk after its group completes so
            # the gelu/split chain hides under the next chunk's PE work
            n_chunks = len(chunks)
            emitted = 0
            for ci in range(n_chunks):
                emit_h_chunk(ci)
                done = (ci + 1) // GCH   # groups fully computed so far
                if (ci + 1) % GCH == 1 and emitted < done:
                    emit_y_group(emitted)
                    emitted += 1
            while emitted < n_groups:
                emit_y_group(emitted)
                emitted += 1

    nc.finalize()
    return nc


def _split_fp8(a, scale, np_fp8):
    s = (a.astype(np.float32) * np.float32(scale))
    hi = np.clip(s, -FP8MAX, FP8MAX).astype(np_fp8)
    lo = np.clip(s - hi.astype(np.float32), -FP8MAX, FP8MAX).astype(np_fp8)
    return hi, lo


def _route(x2d, rw):
    """f64 router: softmax + top-2 (ties -> lower index, like lax.top_k)."""
    logits = x2d.astype(np.float64) @ rw.astype(np.float64)
    m = logits.max(axis=-1, keepdims=True)
    p = np.exp(logits - m)
    p /= p.sum(axis=-1, keepdims=True)
    idx = np.argsort(-p, axis=-1, kind="stable")[:, :TOP_K]
    wts = np.take_along_axis(p, idx, axis=1)
    return idx.astype(np.int64), wts.astype(np.float32)


def make_in_maps(inputs, idx, C):
    import ml_dtypes

    np_fp8 = ml_dtypes.float8_e4m3

    x = np.asarray(inputs["x"], dtype=np.float32).reshape(T, D)
    w1 = np.asarray(inputs["w1"], dtype=np.float32)
    v1 = np.asarray(inputs["v1"], dtype=np.float32)
    w2 = np.asarray(inputs["w2"], dtype=np.float32)

    in_maps = []
    toks = []
    for e in range(E):
        tok = np.where((idx == e).any(axis=1))[0]
        toks.append(tok)
        CP = ((C + P - 1) // P) * P
        xg = np.zeros((CP, D), dtype=np.float32)
        xg[: len(tok)] = x[tok]
        xh, xl = _split_fp8(xg.T, SX, np_fp8)            # [D, CP]
        x8 = np.ascontiguousarray(
            np.stack([xh, xl]).reshape(2, DO, P, CP)
        )
        w1h, w1l = _split_fp8(w1[e], SW, np_fp8)         # [D, F]
        v1h, v1l = _split_fp8(v1[e], SW, np_fp8)
        w2h, w2l = _split_fp8(w2[e], SW2, np_fp8)        # [F, D]
        in_maps.append(
            {
                "x8": x8,
                "w1": np.ascontiguousarray(np.stack([w1h, w1l]).reshape(2, DO, P, F)),
                "v1": np.ascontiguousarray(np.stack([v1h, v1l]).reshape(2, DO, P, F)),
                "w2": np.ascontiguousarray(np.stack([w2h, w2l]).reshape(2, NU, P, D)),
            }
        )
    return in_maps, toks


_NC_CACHE = {}
last_results = None


def kernel(**inputs) -> np.ndarray:
    global last_results
    from concourse.bass_utils import run_bass_kernel_spmd

    x2d = np.asarray(inputs["x"], dtype=np.float32).reshape(T, D)
    rw = np.asarray(inputs["router_w"], dtype=np.float32)
    bias = np.asarray(inputs["bias"], dtype=np.float32)

    idx, wts = _route(x2d, rw)
    counts = np.bincount(idx.ravel(), minlength=E)
    C = int(counts.max())

    key = ("nc", C)
    if key not in _NC_CACHE:
        _NC_CACHE[key] = build_nc(C)
        _NC_CACHE["nc"] = _NC_CACHE[key]
    nc = _NC_CACHE[key]

    in_maps, toks = make_in_maps(inputs, idx, C)
    trace = bool(int(os.environ.get("MOE_TRACE", "0")))
    res = run_bass_kernel_spmd(
        nc, in_maps, core_ids=list(range(E)), trace=trace,
        stitch_traces=trace, trace_cores=list(range(E)) if trace else None,
    )
    last_results = res

    descale = np.float32(1.0 / (SH * SW2))
    out = np.zeros((T, D), dtype=np.float32)
    for e in range(E):
        tok = toks[e]
        yc = np.asarray(res.results[e]["yc"]).astype(np.float32)[: len(tok)]
        we = np.where(idx[tok, 0] == e, wts[tok, 0], wts[tok, 1])
        out[tok] += we[:, None] * (yc * descale)
    out += bias
    return out.reshape(2, 2048, D)
